# revision 23
# baseline (speedup 1.0000x reference)
"""JukeboxAttention Trainium2 kernel (bf16).

Shards the B*BLOCKS=32 independent attention blocks across 8 NeuronCores
(4 blocks = 2048 tokens per core); weights and x are pre-tiled/transposed
on the host for contiguous DMA.

Loop structure is head-outer so each head's c_attn weight slices are
DMA'd once per pass and reused for all 4 blocks (4x less weight traffic
than block-outer); x^T tiles for all 4 blocks and the per-head ctx^T
accumulator stay resident in SBUF. Per (head, block) unit:
  per-head q/k/v in [hd, tok] layout straight from x^T, causal block
  attention in [k, q] layout with triangular tile skipping; the softmax
  denominator rides along the ctx matmul as a ones-column appended to
  v^T (PSUM out [q, 129], col 128 = sum_k probs), so no separate
  denominator matmuls; normalization fused into the PSUM->SBUF copy
  (per-partition Act scale); ctx transposed to [hd, tok] on the PE.
Then a c_proj phase: out = ctx @ c_proj_w + b, written bf16.
"""

import sys

sys.path.insert(0, "/opt/trn_rl_repo")

import numpy as np

B, L, E = 2, 8192, 2048
HEADS, HD = 16, 128
BLOCKS, BC = 16, 512
SCALE2 = float(HD) ** -0.5  # (hd^-0.25)^2 applied to q side
NCORES = 8
BPC = B * BLOCKS // NCORES  # blocks per core = 4
T = BPC * BC  # tokens per core = 2048
ET = E // 128  # 16 contraction tiles


def _build_nc(reps=1, loop=1):
    import concourse.bass as bass  # noqa: F401
    from concourse import bacc, mybir, tile

    f32 = mybir.dt.float32
    bf16 = mybir.dt.bfloat16
    Act = mybir.ActivationFunctionType

    nc = bacc.Bacc("TRN2", target_bir_lowering=False, debug=False)

    # xst[et, p, t] = x[t, et*128+p]  (x^T, tiled over contraction dim)
    xst = nc.dram_tensor("xst", [ET, 128, T], bf16, kind="ExternalInput").ap()
    # waq_t[ft, p, et*128+j] = c_attn_w[et*128+p, ft*128+j]; ft: q=h, k=16+h, v=32+h
    waq = nc.dram_tensor("waq", [3 * ET, 128, E], bf16, kind="ExternalInput").ap()
    cab = nc.dram_tensor("cab", [128, 3 * ET], f32, kind="ExternalInput").ap()
    # wp_t[f, dg, p, dtl*512+j] = c_proj_w[dg*512+dtl*128+p, f*512+j]
    wp = nc.dram_tensor("wp", [4, 4, 128, E], bf16, kind="ExternalInput").ap()
    cpb = nc.dram_tensor("cpb", [E], f32, kind="ExternalInput").ap()
    maskt = nc.dram_tensor("maskt", [128, 128], bf16, kind="ExternalInput").ap()
    ident = nc.dram_tensor("ident", [128, 128], bf16, kind="ExternalInput").ap()
    out = nc.dram_tensor("out", [T, E], bf16, kind="ExternalOutput").ap()

    def emit_body(pools, consts):
        (xtp, wload, ctxp, qkvp, vhp, prp, csp, rcp, outp,
         psA, psC, psT) = pools
        (ident_sb, mask_sb, cab_sb, pbias_bc) = consts

        # ---- x^T tiles for all 4 blocks; 4 chunk-tiles per block so the
        # first matmuls only wait on a 512KB chunk, not the full 2MB.
        # blocks 0/2 stream on gpsimd, 1/3 on sync (between weight loads)
        # so all four arrive before their first consumer ----
        xts = []  # xts[blk][c] covers et = 4c .. 4c+3
        for blk in range(BPC):
            chunks = []
            for c in range(4):
                Xc = xtp.tile([128, 4, BC], bf16, tag="xt")
                chunks.append(Xc)
            xts.append(chunks)

        def load_x(blk, eng):
            t0 = blk * BC
            for c in range(4):
                eng.dma_start(
                    out=xts[blk][c],
                    in_=xst.rearrange("et p t -> p et t")[:, 4 * c:4 * c + 4,
                                                          t0:t0 + BC])

        load_x(0, nc.gpsimd)
        load_x(2, nc.gpsimd)

        ctxTs = []
        for _ in range(BPC):
            ctxT_b = ctxp.tile([128, HEADS, BC], bf16, tag="ctxt")
            ctxTs.append(ctxT_b)

        def qkv_group(w, Xt, bias, scale, copy):
            ps = psA.tile([128, BC], f32, tag="psa")
            for et in range(ET):
                nc.tensor.matmul(ps, lhsT=w[:, et, :], rhs=Xt[et // 4][:, et % 4, :],
                                 start=(et == 0), stop=(et == ET - 1))
            sb = qkvp.tile([128, BC], bf16, tag="qkv")
            if copy == "act":
                nc.scalar.activation(sb, ps, Act.Identity, bias=bias, scale=scale)
            else:
                nc.vector.tensor_scalar_add(sb, ps, bias)
            return sb

        def stage1(h, blk, wq, wk, wv):
            """qkv projection, v^T, scores+exp for one (head, block) unit."""
            Xt = xts[blk]
            v_sb = qkv_group(wv, Xt, cab_sb[:, 2 * ET + h:2 * ET + h + 1], 1.0, "dve")
            q_sb = qkv_group(wq, Xt, cab_sb[:, h:h + 1], SCALE2, "act")
            k_sb = qkv_group(wk, Xt, cab_sb[:, ET + h:ET + h + 1], 1.0, "dve")

            # v^T -> v_head [k, (kt, hd+1)]; col 128 = ones (denominator)
            ps_vt = psT.tile([128, 4 * 128], bf16, tag="pst")
            for kt in range(4):
                nc.tensor.transpose(ps_vt[:, kt * 128:(kt + 1) * 128],
                                    v_sb[:, kt * 128:(kt + 1) * 128], ident_sb)
            v_head = vhp.tile([128, 4, 129], bf16, tag="vh")
            nc.vector.memset(v_head[:, :, 128:129], 1.0)
            nc.vector.tensor_copy(
                v_head[:, :, 0:128],
                ps_vt.rearrange("p (kt j) -> p kt j", kt=4))

            # scores^T [k, q] -> exp -> diagonal mask; triangular tile skip.
            # s2 (256 cols) and s3 (128 cols) share one PSUM bank so the
            # psA ring wraps onto DVE-freed banks, not Act-freed ones.
            pbs = []
            ps_s23 = None
            for kt in range(4):
                qlen = BC - kt * 128
                if kt < 2:
                    ps_s = psA.tile([128, BC], f32, tag="psa")
                    dst = ps_s[:, :qlen]
                elif kt == 2:
                    ps_s23 = psA.tile([128, BC], f32, tag="psa")
                    dst = ps_s23[:, 0:256]
                else:
                    dst = ps_s23[:, 256:384]
                nc.tensor.matmul(dst, lhsT=k_sb[:, kt * 128:(kt + 1) * 128],
                                 rhs=q_sb[:, kt * 128:], start=True, stop=True)
                pb = prp.tile([128, BC], bf16, tag="pb")
                nc.scalar.activation(pb[:, :qlen], dst, Act.Exp)
                nc.vector.tensor_mul(pb[:, :128], pb[:, :128], mask_sb)
                pbs.append(pb)
            return (h, blk, v_head, pbs)

        def stage2(st):
            """ctx (+fused denominator), normalize, transpose into ctxT."""
            h, blk, v_head, pbs = st
            ps_cs = []
            for _ in range(2):
                ps_cpair = psC.tile([128, 2, 129], f32, tag="psc")
                ps_cs.append(ps_cpair)
            for qt in range(4):
                ps_c = ps_cs[qt // 2][:, qt % 2, :]
                for kt in range(qt + 1):
                    off = (qt - kt) * 128
                    nc.tensor.matmul(ps_c,
                                     lhsT=pbs[kt][:, off:off + 128],
                                     rhs=v_head[:, kt, :],
                                     start=(kt == 0), stop=(kt == qt))
            recip = rcp.tile([128, 4], f32, tag="recip")
            nc.vector.reciprocal(recip[:, 0:2], ps_cs[0][:, :, 128])
            nc.vector.reciprocal(recip[:, 2:4], ps_cs[1][:, :, 128])

            ctx_sb = csp.tile([128, 4, 128], bf16, tag="csb")
            for qt in range(4):
                nc.scalar.activation(ctx_sb[:, qt, :],
                                     ps_cs[qt // 2][:, qt % 2, 0:128], Act.Identity,
                                     scale=recip[:, qt:qt + 1])

            ps_t = psT.tile([128, 4 * 128], bf16, tag="pst")
            for qt in range(4):
                nc.tensor.transpose(ps_t[:, qt * 128:(qt + 1) * 128],
                                    ctx_sb[:, qt, :], ident_sb)
            if h % 2 == 0:
                nc.vector.tensor_copy(ctxTs[blk][:, h, :], ps_t)
            else:
                nc.scalar.copy(ctxTs[blk][:, h, :], ps_t)

        def load_wp(f):
            wpgs = []
            for dg in range(4):
                wpg = wload.tile([128, 4, BC], bf16, tag="w")
                nc.gpsimd.dma_start(out=wpg, in_=wp[f, dg].rearrange("p (dt j) -> p dt j", dt=4))
                wpgs.append(wpg)
            return wpgs

        # ---- phase 1: attention, head-outer (weights loaded once/head),
        # software-pipelined so unit i's ctx chain is emitted after unit
        # i+1's qkv/scores matmuls (PE never waits on the Act/DVE chain) ----
        prev = None
        wp_pre = {}
        for h in range(HEADS):
            wv = wload.tile([128, ET, 128], bf16, tag="w")
            nc.sync.dma_start(out=wv, in_=waq[2 * ET + h].rearrange("p (et j) -> p et j", et=ET))
            wq = wload.tile([128, ET, 128], bf16, tag="w")
            nc.sync.dma_start(out=wq, in_=waq[h].rearrange("p (et j) -> p et j", et=ET))
            wk = wload.tile([128, ET, 128], bf16, tag="w")
            nc.sync.dma_start(out=wk, in_=waq[ET + h].rearrange("p (et j) -> p et j", et=ET))
            if h == 0:
                load_x(1, nc.sync)
            elif h == 1:
                load_x(3, nc.sync)
            elif h == HEADS - 2:
                wp_pre[0] = load_wp(0)  # ring slots died ~h12; lands during h14
            elif h == HEADS - 1:
                wp_pre[1] = load_wp(1)
            for blk in range(BPC):
                st = stage1(h, blk, wq, wk, wv)
                if prev is not None:
                    stage2(prev)
                prev = st
        stage2(prev)

        # ---- phase 2: out = ctx @ c_proj_w + b (bf16 out) ----
        wp_pre[2] = load_wp(2)
        wp_pre[3] = load_wp(3)
        for f in range(4):
            wpgs = wp_pre[f]
            for blk in range(BPC):
                t0 = blk * BC
                ctxT = ctxTs[blk]
                for m in range(4):
                    ps_o = psA.tile([128, BC], f32, tag="psa")
                    for dg in range(4):
                        for dtl in range(4):
                            dt = dg * 4 + dtl
                            nc.tensor.matmul(
                                ps_o,
                                lhsT=ctxT[:, dt, m * 128:(m + 1) * 128],
                                rhs=wpgs[dg][:, dtl, :],
                                start=(dt == 0), stop=(dt == 15),
                            )
                    osb = outp.tile([128, BC], bf16, tag="osb")
                    nc.vector.tensor_add(osb, ps_o, pbias_bc[:, f * 512:(f + 1) * 512])
                    eng = (nc.gpsimd, nc.scalar)[m % 2]
                    eng.dma_start(
                        out=out[t0 + m * 128: t0 + (m + 1) * 128, f * 512:(f + 1) * 512],
                        in_=osb,
                    )

    with tile.TileContext(nc) as tc:
        with (
            tc.tile_pool(name="const", bufs=1) as const,
            tc.tile_pool(name="wload", bufs=10) as wload,
            tc.tile_pool(name="xt", bufs=4 * BPC) as xtp,
            tc.tile_pool(name="ctxt", bufs=BPC) as ctxp,
            tc.tile_pool(name="qkv", bufs=7) as qkvp,
            tc.tile_pool(name="vh", bufs=3) as vhp,
            tc.tile_pool(name="probs", bufs=8) as prp,
            tc.tile_pool(name="csb", bufs=3) as csp,
            tc.tile_pool(name="recip", bufs=2) as rcp,
            tc.tile_pool(name="outp", bufs=6) as outp,
            tc.tile_pool(name="psA", bufs=4, space="PSUM") as psA,
            tc.tile_pool(name="psC", bufs=2, space="PSUM") as psC,
            tc.tile_pool(name="psT", bufs=2, space="PSUM") as psT,  # 4+2+2 = 8 banks
        ):
            # ---- constants ----
            ident_sb = const.tile([128, 128], bf16, tag="ident")
            nc.sync.dma_start(out=ident_sb, in_=ident)
            mask_sb = const.tile([128, 128], bf16, tag="mask")
            nc.sync.dma_start(out=mask_sb, in_=maskt)
            cab_sb = const.tile([128, 3 * ET], f32, tag="cab")
            nc.sync.dma_start(out=cab_sb, in_=cab)
            pbias_bc = const.tile([128, E], f32, tag="pbias")
            pb_ap = bass.AP(tensor=cpb.tensor, offset=cpb.offset,
                            ap=[[0, 128], [1, E]])
            nc.scalar.dma_start(out=pbias_bc, in_=pb_ap)

            pools = (xtp, wload, ctxp, qkvp, vhp, prp, csp, rcp, outp,
                     psA, psC, psT)
            consts = (ident_sb, mask_sb, cab_sb, pbias_bc)

            if loop > 1:
                with tc.For_i(0, loop):
                    for _ in range(reps):
                        emit_body(pools, consts)
            else:
                for _ in range(reps):
                    emit_body(pools, consts)
    nc.compile()
    return nc


_NC = {}


def _get_nc(reps=1, loop=1):
    key = (reps, loop)
    if key not in _NC:
        _NC[key] = _build_nc(reps, loop)
    return _NC[key]


def make_in_maps(x, c_attn_w, c_attn_b, c_proj_w, c_proj_b):
    import ml_dtypes

    bf = ml_dtypes.bfloat16
    x = np.asarray(x, np.float32)
    c_attn_w = np.asarray(c_attn_w, np.float32)
    c_proj_w = np.asarray(c_proj_w, np.float32)
    c_attn_b = np.asarray(c_attn_b, np.float32)
    c_proj_b = np.asarray(c_proj_b, np.float32)

    # waq_t[ft, p, et, j] = c_attn_w[et*128+p, ft*128+j]
    waq_t = np.ascontiguousarray(
        c_attn_w.reshape(ET, 128, 3 * ET, 128).transpose(2, 1, 0, 3)
        .reshape(3 * ET, 128, E).astype(bf))
    # wp_t[f, dg, p, dtl, j] = c_proj_w[dg*512+dtl*128+p, f*512+j]
    wp_t = np.ascontiguousarray(
        c_proj_w.reshape(4, 4, 128, 4, 512).transpose(3, 0, 2, 1, 4)
        .reshape(4, 4, 128, E).astype(bf))
    # q/k/v biases, q side pre-scaled (activation: out = scale*in + bias)
    b_mod = c_attn_b.copy()
    b_mod[:E] *= SCALE2
    cab = np.ascontiguousarray(b_mod.reshape(3 * ET, 128).T)
    # within-diagonal-tile causal mask: col(query) >= row(key)
    p = np.arange(128)[:, None]
    c = np.arange(128)[None, :]
    maskt = np.ascontiguousarray((c >= p).astype(bf))
    ident = np.eye(128, dtype=bf)

    xr = x.reshape(B * BLOCKS, BC, E)
    in_maps = []
    for core in range(NCORES):
        xs = xr[core * BPC:(core + 1) * BPC].reshape(T, E)
        # xst[et, p, t] = xs[t, et*128+p]
        xst = np.ascontiguousarray(
            xs.T.reshape(ET, 128, T).astype(bf))
        in_maps.append({
            "xst": xst, "waq": waq_t, "cab": cab, "wp": wp_t,
            "cpb": c_proj_b, "maskt": maskt, "ident": ident,
        })
    return in_maps


def kernel(x, c_attn_w, c_attn_b, c_proj_w, c_proj_b):
    from concourse import bass_utils

    nc = _get_nc()
    in_maps = make_in_maps(x, c_attn_w, c_attn_b, c_proj_w, c_proj_b)
    res = bass_utils.run_bass_kernel_spmd(nc, in_maps, core_ids=list(range(NCORES)))
    outs = [np.asarray(res.results[c]["out"], np.float32) for c in range(NCORES)]
    full = np.concatenate(outs, axis=0).reshape(B, L, E).astype(np.float32)
    return full


# revision 28
# speedup vs baseline: 1.0076x; 1.0076x over previous
"""JukeboxAttention Trainium2 kernel (bf16).

Shards the B*BLOCKS=32 independent attention blocks across 8 NeuronCores
(4 blocks = 2048 tokens per core); weights and x are pre-tiled/transposed
on the host for contiguous DMA.

Loop structure is head-outer so each head's c_attn weight slices are
DMA'd once per pass and reused for all 4 blocks (4x less weight traffic
than block-outer); x^T tiles for all 4 blocks and the per-block ctx^T
accumulators stay resident in SBUF. Per (head, block) unit:
  per-head q/k/v in [hd, tok] layout straight from x^T (q copy on Act
  with fused bias+scale, k/v copies on DVE so PSUM banks free in
  parallel), causal block attention in [k, q] layout with triangular
  tile skipping; the softmax denominator rides along the ctx matmul as
  a ones-column appended to v^T (PSUM out [q, 129], col 128 = sum_k
  probs), so no separate denominator matmuls; normalization fused into
  the PSUM->SBUF copy (per-partition Act scale); ctx transposed to
  [hd, tok] on the PE.
The emission is software-pipelined two units deep: unit i's ctx chain
is emitted after unit i+1's qkv/scores matmuls so the PE never waits on
the Act/DVE probs chain. A final c_proj phase (weights prefetched into
the ring during the last heads) computes out = ctx @ c_proj_w + b in
bf16.
"""

import sys

sys.path.insert(0, "/opt/trn_rl_repo")

import numpy as np

B, L, E = 2, 8192, 2048
HEADS, HD = 16, 128
BLOCKS, BC = 16, 512
SCALE2 = float(HD) ** -0.5  # (hd^-0.25)^2 applied to q side
NCORES = 8
BPC = B * BLOCKS // NCORES  # blocks per core = 4
T = BPC * BC  # tokens per core = 2048
ET = E // 128  # 16 contraction tiles


def _build_nc(reps=1, loop=1):
    import concourse.bass as bass  # noqa: F401
    from concourse import bacc, mybir, tile

    f32 = mybir.dt.float32
    bf16 = mybir.dt.bfloat16
    Act = mybir.ActivationFunctionType

    nc = bacc.Bacc("TRN2", target_bir_lowering=False, debug=False)

    # xst[et, p, t] = x[t, et*128+p]  (x^T, tiled over contraction dim)
    xst = nc.dram_tensor("xst", [ET, 128, T], bf16, kind="ExternalInput").ap()
    # waq_t[ft, p, et*128+j] = c_attn_w[et*128+p, ft*128+j]; ft: q=h, k=16+h, v=32+h
    waq = nc.dram_tensor("waq", [3 * ET, 128, E], bf16, kind="ExternalInput").ap()
    cab = nc.dram_tensor("cab", [128, 3 * ET], f32, kind="ExternalInput").ap()
    # wp_t[f, dg, p, dtl*512+j] = c_proj_w[dg*512+dtl*128+p, f*512+j]
    wp = nc.dram_tensor("wp", [4, 4, 128, E], bf16, kind="ExternalInput").ap()
    cpb = nc.dram_tensor("cpb", [E], f32, kind="ExternalInput").ap()
    maskt = nc.dram_tensor("maskt", [128, 128], bf16, kind="ExternalInput").ap()
    ident = nc.dram_tensor("ident", [128, 128], bf16, kind="ExternalInput").ap()
    out = nc.dram_tensor("out", [T, E], bf16, kind="ExternalOutput").ap()

    def emit_body(pools, consts):
        (xtp, wload, ctxp, qkvp, vhp, prp, csp, rcp, outp,
         psA, psC, psT) = pools
        (ident_sb, mask_sb, cab_sb, pbias_bc) = consts

        # ---- x^T tiles for all 4 blocks; 4 chunk-tiles per block so the
        # first matmuls only wait on a 512KB chunk, not the full 2MB.
        # blocks 0/2 stream on gpsimd, 1/3 on sync (between weight loads)
        # so all four arrive before their first consumer ----
        xts = []  # xts[blk][c] covers et = 4c .. 4c+3
        for blk in range(BPC):
            chunks = []
            for c in range(4):
                Xc = xtp.tile([128, 4, BC], bf16, tag="xt")
                chunks.append(Xc)
            xts.append(chunks)

        def load_x(blk, eng):
            t0 = blk * BC
            for c in range(4):
                eng.dma_start(
                    out=xts[blk][c],
                    in_=xst.rearrange("et p t -> p et t")[:, 4 * c:4 * c + 4,
                                                          t0:t0 + BC])

        load_x(0, nc.gpsimd)
        load_x(2, nc.gpsimd)

        ctxTs = []
        for _ in range(BPC):
            ctxT_b = ctxp.tile([128, HEADS, BC], bf16, tag="ctxt")
            ctxTs.append(ctxT_b)

        def qkv_group(w, Xt, bias, scale, copy):
            ps = psA.tile([128, BC], f32, tag="psa")
            for et in range(ET):
                nc.tensor.matmul(ps, lhsT=w[:, et, :], rhs=Xt[et // 4][:, et % 4, :],
                                 start=(et == 0), stop=(et == ET - 1))
            sb = qkvp.tile([128, BC], bf16, tag="qkv")
            if copy == "act":
                nc.scalar.activation(sb, ps, Act.Identity, bias=bias, scale=scale)
            else:
                nc.vector.tensor_scalar_add(sb, ps, bias)
            return sb

        def stage1(h, blk, wq, wk, wv):
            """qkv projection, v^T, scores+exp for one (head, block) unit."""
            Xt = xts[blk]
            v_sb = qkv_group(wv, Xt, cab_sb[:, 2 * ET + h:2 * ET + h + 1], 1.0, "dve")
            q_sb = qkv_group(wq, Xt, cab_sb[:, h:h + 1], SCALE2, "act")
            k_sb = qkv_group(wk, Xt, cab_sb[:, ET + h:ET + h + 1], 1.0, "dve")

            # v^T -> v_head [k, (kt, hd+1)]; col 128 = ones (denominator)
            ps_vt = psT.tile([128, 4 * 128], bf16, tag="pst")
            for kt in range(4):
                nc.tensor.transpose(ps_vt[:, kt * 128:(kt + 1) * 128],
                                    v_sb[:, kt * 128:(kt + 1) * 128], ident_sb)
            v_head = vhp.tile([128, 4, 129], bf16, tag="vh")
            nc.vector.memset(v_head[:, :, 128:129], 1.0)
            nc.vector.tensor_copy(
                v_head[:, :, 0:128],
                ps_vt.rearrange("p (kt j) -> p kt j", kt=4))

            # scores^T [k, q] -> exp -> diagonal mask; triangular tile skip.
            # s2 (256 cols) and s3 (128 cols) share one PSUM bank so the
            # psA ring wraps onto DVE-freed banks, not Act-freed ones.
            pbs = []
            ps_s23 = None
            for kt in range(4):
                qlen = BC - kt * 128
                if kt < 2:
                    ps_s = psA.tile([128, BC], f32, tag="psa")
                    dst = ps_s[:, :qlen]
                elif kt == 2:
                    ps_s23 = psA.tile([128, BC], f32, tag="psa")
                    dst = ps_s23[:, 0:256]
                else:
                    dst = ps_s23[:, 256:384]
                nc.tensor.matmul(dst, lhsT=k_sb[:, kt * 128:(kt + 1) * 128],
                                 rhs=q_sb[:, kt * 128:], start=True, stop=True)
                pb = prp.tile([128, BC], bf16, tag="pb")
                nc.scalar.activation(pb[:, :qlen], dst, Act.Exp)
                nc.vector.tensor_mul(pb[:, :128], pb[:, :128], mask_sb)
                pbs.append(pb)
            return (h, blk, v_head, pbs)

        def stage2(st):
            """ctx (+fused denominator), normalize, transpose into ctxT."""
            h, blk, v_head, pbs = st
            ps_cs = []
            for _ in range(2):
                ps_cpair = psC.tile([128, 2, 129], f32, tag="psc")
                ps_cs.append(ps_cpair)
            for qt in range(4):
                ps_c = ps_cs[qt // 2][:, qt % 2, :]
                for kt in range(qt + 1):
                    off = (qt - kt) * 128
                    nc.tensor.matmul(ps_c,
                                     lhsT=pbs[kt][:, off:off + 128],
                                     rhs=v_head[:, kt, :],
                                     start=(kt == 0), stop=(kt == qt))
            recip = rcp.tile([128, 4], f32, tag="recip")
            nc.vector.reciprocal(recip[:, 0:2], ps_cs[0][:, :, 128])
            nc.vector.reciprocal(recip[:, 2:4], ps_cs[1][:, :, 128])

            ctx_sb = csp.tile([128, 4, 128], bf16, tag="csb")
            for qt in range(4):
                nc.scalar.activation(ctx_sb[:, qt, :],
                                     ps_cs[qt // 2][:, qt % 2, 0:128], Act.Identity,
                                     scale=recip[:, qt:qt + 1])

            ps_t = psT.tile([128, 4 * 128], bf16, tag="pst")
            for qt in range(4):
                nc.tensor.transpose(ps_t[:, qt * 128:(qt + 1) * 128],
                                    ctx_sb[:, qt, :], ident_sb)
            if h % 2 == 0:
                nc.vector.tensor_copy(ctxTs[blk][:, h, :], ps_t)
            else:
                nc.scalar.copy(ctxTs[blk][:, h, :], ps_t)

        def load_wp(f):
            wpgs = []
            for dg in range(4):
                wpg = wload.tile([128, 4, BC], bf16, tag="w")
                nc.gpsimd.dma_start(out=wpg, in_=wp[f, dg].rearrange("p (dt j) -> p dt j", dt=4))
                wpgs.append(wpg)
            return wpgs

        # ---- phase 1: attention, head-outer (weights loaded once/head),
        # software-pipelined so unit i's ctx chain is emitted after unit
        # i+1's qkv/scores matmuls (PE never waits on the Act/DVE chain) ----
        prev = None
        wp_pre = {}
        for h in range(HEADS):
            wv = wload.tile([128, ET, 128], bf16, tag="w")
            nc.sync.dma_start(out=wv, in_=waq[2 * ET + h].rearrange("p (et j) -> p et j", et=ET))
            wq = wload.tile([128, ET, 128], bf16, tag="w")
            nc.sync.dma_start(out=wq, in_=waq[h].rearrange("p (et j) -> p et j", et=ET))
            wk = wload.tile([128, ET, 128], bf16, tag="w")
            nc.sync.dma_start(out=wk, in_=waq[ET + h].rearrange("p (et j) -> p et j", et=ET))
            if h == 0:
                # both must be emitted before any stage1 that reads them
                load_x(1, nc.sync)
                load_x(3, nc.sync)
            elif h == HEADS - 2:
                wp_pre[0] = load_wp(0)  # ring slots died ~h12; lands during h14
            elif h == HEADS - 1:
                wp_pre[1] = load_wp(1)
            for blk in range(BPC):
                st = stage1(h, blk, wq, wk, wv)
                if prev is not None:
                    stage2(prev)
                prev = st
        stage2(prev)

        # ---- phase 2: out = ctx @ c_proj_w + b (bf16 out) ----
        wp_pre[2] = load_wp(2)
        wp_pre[3] = load_wp(3)
        for f in range(4):
            wpgs = wp_pre[f]
            for blk in range(BPC):
                t0 = blk * BC
                ctxT = ctxTs[blk]
                for m in range(4):
                    ps_o = psA.tile([128, BC], f32, tag="psa")
                    for dg in range(4):
                        for dtl in range(4):
                            dt = dg * 4 + dtl
                            nc.tensor.matmul(
                                ps_o,
                                lhsT=ctxT[:, dt, m * 128:(m + 1) * 128],
                                rhs=wpgs[dg][:, dtl, :],
                                start=(dt == 0), stop=(dt == 15),
                            )
                    osb = outp.tile([128, BC], bf16, tag="osb")
                    nc.vector.tensor_add(osb, ps_o, pbias_bc[:, f * 512:(f + 1) * 512])
                    eng = (nc.gpsimd, nc.scalar)[m % 2]
                    eng.dma_start(
                        out=out[t0 + m * 128: t0 + (m + 1) * 128, f * 512:(f + 1) * 512],
                        in_=osb,
                    )

    with tile.TileContext(nc) as tc:
        with (
            tc.tile_pool(name="const", bufs=1) as const,
            tc.tile_pool(name="wload", bufs=10) as wload,
            tc.tile_pool(name="xt", bufs=4 * BPC) as xtp,
            tc.tile_pool(name="ctxt", bufs=BPC) as ctxp,
            tc.tile_pool(name="qkv", bufs=7) as qkvp,
            tc.tile_pool(name="vh", bufs=3) as vhp,
            tc.tile_pool(name="probs", bufs=8) as prp,
            tc.tile_pool(name="csb", bufs=3) as csp,
            tc.tile_pool(name="recip", bufs=2) as rcp,
            tc.tile_pool(name="outp", bufs=4) as outp,
            tc.tile_pool(name="psA", bufs=4, space="PSUM") as psA,
            tc.tile_pool(name="psC", bufs=2, space="PSUM") as psC,
            tc.tile_pool(name="psT", bufs=2, space="PSUM") as psT,  # 4+2+2 = 8 banks
        ):
            # ---- constants ----
            ident_sb = const.tile([128, 128], bf16, tag="ident")
            nc.sync.dma_start(out=ident_sb, in_=ident)
            mask_sb = const.tile([128, 128], bf16, tag="mask")
            nc.sync.dma_start(out=mask_sb, in_=maskt)
            cab_sb = const.tile([128, 3 * ET], f32, tag="cab")
            nc.sync.dma_start(out=cab_sb, in_=cab)
            pbias_bc = const.tile([128, E], f32, tag="pbias")
            pb_ap = bass.AP(tensor=cpb.tensor, offset=cpb.offset,
                            ap=[[0, 128], [1, E]])
            nc.scalar.dma_start(out=pbias_bc, in_=pb_ap)

            pools = (xtp, wload, ctxp, qkvp, vhp, prp, csp, rcp, outp,
                     psA, psC, psT)
            consts = (ident_sb, mask_sb, cab_sb, pbias_bc)

            if loop > 1:
                with tc.For_i(0, loop):
                    for _ in range(reps):
                        emit_body(pools, consts)
            else:
                for _ in range(reps):
                    emit_body(pools, consts)
    nc.compile()
    return nc


_NC = {}


def _get_nc(reps=1, loop=1):
    key = (reps, loop)
    if key not in _NC:
        _NC[key] = _build_nc(reps, loop)
    return _NC[key]


def make_in_maps(x, c_attn_w, c_attn_b, c_proj_w, c_proj_b):
    import ml_dtypes

    bf = ml_dtypes.bfloat16
    x = np.asarray(x, np.float32)
    c_attn_w = np.asarray(c_attn_w, np.float32)
    c_proj_w = np.asarray(c_proj_w, np.float32)
    c_attn_b = np.asarray(c_attn_b, np.float32)
    c_proj_b = np.asarray(c_proj_b, np.float32)

    # waq_t[ft, p, et, j] = c_attn_w[et*128+p, ft*128+j]
    waq_t = np.ascontiguousarray(
        c_attn_w.reshape(ET, 128, 3 * ET, 128).transpose(2, 1, 0, 3)
        .reshape(3 * ET, 128, E).astype(bf))
    # wp_t[f, dg, p, dtl, j] = c_proj_w[dg*512+dtl*128+p, f*512+j]
    wp_t = np.ascontiguousarray(
        c_proj_w.reshape(4, 4, 128, 4, 512).transpose(3, 0, 2, 1, 4)
        .reshape(4, 4, 128, E).astype(bf))
    # q/k/v biases, q side pre-scaled (activation: out = scale*in + bias)
    b_mod = c_attn_b.copy()
    b_mod[:E] *= SCALE2
    cab = np.ascontiguousarray(b_mod.reshape(3 * ET, 128).T)
    # within-diagonal-tile causal mask: col(query) >= row(key)
    p = np.arange(128)[:, None]
    c = np.arange(128)[None, :]
    maskt = np.ascontiguousarray((c >= p).astype(bf))
    ident = np.eye(128, dtype=bf)

    xr = x.reshape(B * BLOCKS, BC, E)
    in_maps = []
    for core in range(NCORES):
        xs = xr[core * BPC:(core + 1) * BPC].reshape(T, E)
        # xst[et, p, t] = xs[t, et*128+p]
        xst = np.ascontiguousarray(
            xs.T.reshape(ET, 128, T).astype(bf))
        in_maps.append({
            "xst": xst, "waq": waq_t, "cab": cab, "wp": wp_t,
            "cpb": c_proj_b, "maskt": maskt, "ident": ident,
        })
    return in_maps


def kernel(x, c_attn_w, c_attn_b, c_proj_w, c_proj_b):
    from concourse import bass_utils

    nc = _get_nc()
    in_maps = make_in_maps(x, c_attn_w, c_attn_b, c_proj_w, c_proj_b)
    res = bass_utils.run_bass_kernel_spmd(nc, in_maps, core_ids=list(range(NCORES)))
    outs = [np.asarray(res.results[c]["out"], np.float32) for c in range(NCORES)]
    full = np.concatenate(outs, axis=0).reshape(B, L, E).astype(np.float32)
    return full
